# revision 15
# baseline (speedup 1.0000x reference)
"""AttentionWithSelfAblation TRN2 kernel.

Reference computation (B=4, S=2048, H=1024, nh=16, hd=64, window=256):
    q = x @ Wq.T ; k = x_clean @ Wk.T ; v = x_clean @ Wv.T   (per-head split)
    scores = q @ k.T  (NO 1/sqrt(hd) scaling)
    local causal mask: key j visible to query i iff i-255 <= j <= i
    attn = softmax(scores) ; ctx = attn @ v  (merge heads)
    out = (ctx * ablation_mask) @ Wo.T + bo
Sharding: pure data/sequence parallel over 8 cores: core c = (batch c//2,
sequence half c%2 of 1024 queries). Keys/values need a 256-halo to the left;
the first half uses zero-padding + masks instead. No collectives.

Dtypes: fp16 for x/xc/weights/qT/kT/ctx (scores accumulate fp32 in PSUM),
bf16 for v/exp (exp needs bf16 range: raw scores reach ~75; exp(s-20) bias
cancels in the softmax normalization). Measured end-to-end rel err ~3e-3.

Per-core device pipeline (all feature-major "T" layouts):
  phase Q : xT chunks streamed -> qT[o,s]
  phase KV: xcT chunks streamed -> kT[o,s] + v[s,o] (o augmented with a
            ones column per head: ctx matmul also yields the denominator)
  phase A : per (qpair of 256 queries, head pair): raw scoresT[sk,sq] by
            interleaved row-disjoint 64-row qk MMs (PE runs pairs
            concurrently); exp(s-20) on ACT -> bf16; {0,1} band mask
            multiply on DVE; ctx MMs split into 64-row halves into two
            PSUM banks (A/B) interleaved for PE concurrency; den =
            A[64]+B[64] -> ones-MM broadcast to 64 partitions -> DVE
            reciprocal; drain = (A+B) then *recip then *ablation -> fp16
            ctx; out-proj (PE, fp16) + bias (ACT).
Host does all layout transposes (free) and unshards by concatenation.
"""

import numpy as np
import ml_dtypes

from concourse import bacc
import concourse.tile as tile
import concourse.mybir as mybir
from concourse.bass_utils import run_bass_kernel_spmd

B, S, H = 4, 2048, 1024
NH, HD = 16, 64
W = 256  # window
SL = 1024  # per-core sequence chunk
SKL = SL + W  # keys incl halo
NQP = SL // 256  # qpairs of 256 queries
NKT = 4  # k-tiles of 128 per qpair
NC = 8  # cores

F32 = mybir.dt.float32
F32R = mybir.dt.float32r
F16 = mybir.dt.float16
BF16 = mybir.dt.bfloat16
EXP = mybir.ActivationFunctionType.Exp
IDENT = mybir.ActivationFunctionType.Identity
MULT = mybir.AluOpType.mult

EXP_BIAS = -20.0  # exp(s + EXP_BIAS): cancels in softmax, avoids overflow

_compiled = None


def _build():
    nc = bacc.Bacc("TRN2", target_bir_lowering=False, debug=False)

    xT = nc.dram_tensor("xT", [H, SL], F16, kind="ExternalInput")
    xcT = nc.dram_tensor("xcT", [H, SKL], F16, kind="ExternalInput")
    ablT = nc.dram_tensor("ablT", [H, SL], F16, kind="ExternalInput")
    WqT = nc.dram_tensor("WqT", [H, H], F16, kind="ExternalInput")
    WkT = nc.dram_tensor("WkT", [H, H], F16, kind="ExternalInput")
    WvT = nc.dram_tensor("WvT", [H, H], F16, kind="ExternalInput")
    WoT = nc.dram_tensor("WoT", [H, H], F16, kind="ExternalInput")
    bo = nc.dram_tensor("bo", [H], F32, kind="ExternalInput")
    # masks[set, kt, sk, sq] in {1,0}: set 0 = standard, set 1 = qp==0 variant
    masks = nc.dram_tensor("masks", [2, NKT, 128, 256], BF16, kind="ExternalInput")
    pmask_in = nc.dram_tensor("pmask_in", [1, 2, 128], BF16, kind="ExternalInput")
    outT = nc.dram_tensor("outT", [H, SL], F32, kind="ExternalOutput")

    xT_d = xT.rearrange("(c p) s -> p c s", p=128)
    xcT_d = xcT.rearrange("(c p) s -> p c s", p=128)
    ablT_d = ablT.rearrange("(t p) s -> p t s", p=128)
    outT_d = outT.rearrange("(t p) s -> p t s", p=128)

    with tile.TileContext(nc) as tc:
        with (
            tc.tile_pool(name="consts", bufs=1) as consts,
            tc.tile_pool(name="big", bufs=1) as big,
            tc.tile_pool(name="wpool", bufs=4) as wpool,
            tc.tile_pool(name="outp", bufs=3) as outpool,
            tc.tile_pool(name="ps512", bufs=2, space="PSUM") as ps512,
        ):

            qT_sb = big.tile([128, 8, SL], F16)
            kT_sb = big.tile([128, 8, SKL], F16)
            v_sb = big.tile([128, 10, 16 * 65], BF16)

            def load_weight_half(dram, hf):
                """o-columns [hf*512, (hf+1)*512) of a transposed weight."""
                w_sb = wpool.tile(
                    [128, 8, 512], F16, name=f"w_{dram.name}_{hf}", tag="w"
                )
                for c in range(8):
                    nc.sync.dma_start(
                        w_sb[:, c, :],
                        dram.rearrange("(c p) o -> p c o", p=128)[
                            :, c, hf * 512 : (hf + 1) * 512
                        ],
                    )
                return w_sb

            # ones columns of the augmented v (slot 64 of each head's 65):
            # memset a contiguous scratch then strided-copy into place
            v_aug = v_sb[:].rearrange("p t (h e) -> p t h e", e=65)
            ones_scratch = consts.tile([128, 160], BF16)
            nc.vector.memset(ones_scratch[:], 1.0)
            nc.vector.tensor_copy(
                v_aug[:, :, :, 64],
                ones_scratch[:].rearrange("p (t h) -> p t h", t=10),
            )
            # per-partition bias column for exp(s + EXP_BIAS)
            ebias = consts.tile([128, 1], F32)
            nc.vector.memset(ebias[:], EXP_BIAS)

            # ---- phase Q with scoped projection PSUM pool ----
            # All weight halves fit in SBUF at fp16 (bufs=8): prefetch each
            # at least one phase ahead so no phase boundary waits on DMA.
            wq_hs = [load_weight_half(WqT, hf) for hf in range(2)]
            wk_hs = [None, None]
            wv_hs = [None, None]
            wo_hs = [None, None]
            with tc.tile_pool(name="xs", bufs=2) as xspool:
                for hf in range(2):
                    wq_sb = wq_hs[hf]
                    for ci in range(SL // 512):
                        x_s = xspool.tile(
                            [128, 8, 512], F16, name=f"x_{hf}_{ci}", tag="xs"
                        )
                        for c in range(8):
                            nc.sync.dma_start(
                                x_s[:, c, :], xT_d[:, c, ci * 512 : (ci + 1) * 512]
                            )
                        for oi in range(4):
                            ot = hf * 4 + oi
                            ps = ps512.tile([128, 512], F32, tag="ps512")
                            for c in range(8):
                                nc.tensor.matmul(
                                    ps[:],
                                    wq_sb[:, c, oi * 128 : (oi + 1) * 128],
                                    x_s[:, c, :],
                                    start=(c == 0),
                                    stop=(c == 7),
                                )
                            nc.vector.tensor_copy(
                                qT_sb[:, ot, ci * 512 : (ci + 1) * 512], ps[:]
                            )
                        # stagger the remaining weight prefetches across the
                        # phase-Q chunks (emission order drives DMA issue)
                        if hf == 0 and ci == 0:
                            wk_hs[0] = load_weight_half(WkT, 0)
                            wv_hs[0] = load_weight_half(WvT, 0)
                        elif hf == 1 and ci == 0:
                            wk_hs[1] = load_weight_half(WkT, 1)
                            wv_hs[1] = load_weight_half(WvT, 1)
                        elif hf == 1 and ci == 1:
                            wo_hs[0] = load_weight_half(WoT, 0)
                            wo_hs[1] = load_weight_half(WoT, 1)

            # ---- attention constants: emitted between phases so their DMAs
            # overlap phase-KV compute and are resident before phase A ----
            bo_sb = consts.tile([128, 8], F32)
            nc.sync.dma_start(bo_sb[:], bo.rearrange("(t p) -> p t", p=128))
            pmask = consts.tile([1, 2, 128], BF16)
            nc.sync.dma_start(pmask[:], pmask_in[:])
            mask_sb = consts.tile([128, 2, NKT, 256], BF16)
            for ms in range(2):
                nc.sync.dma_start(
                    mask_sb[:, ms, :, :],
                    masks.rearrange("s t k q -> k s t q")[:, ms, :, :],
                )

            # ---- phase KV: kT[o, s] + v[s, o] (o augmented per head) ----
            kv_chunks = [(0, 512), (512, 512), (1024, 256)]
            with tc.tile_pool(name="xcs", bufs=2) as xcspool:
                for hf in range(2):
                    wk_sb = wk_hs[hf]
                    wv_sb = wv_hs[hf]
                    for ci, (s0c, snc) in enumerate(kv_chunks):
                        xc_s = xcspool.tile(
                            [128, 8, 512], F16, name=f"xc_{hf}_{ci}", tag="xcs"
                        )
                        for c in range(8):
                            nc.sync.dma_start(
                                xc_s[:, c, :snc], xcT_d[:, c, s0c : s0c + snc]
                            )
                        for oi in range(4):
                            ot = hf * 4 + oi
                            ps = ps512.tile([128, 512], F32, tag="ps512")
                            for c in range(8):
                                nc.tensor.matmul(
                                    ps[:, :snc],
                                    wk_sb[:, c, oi * 128 : (oi + 1) * 128],
                                    xc_s[:, c, :snc],
                                    start=(c == 0),
                                    stop=(c == 7),
                                )
                            nc.vector.tensor_copy(
                                kT_sb[:, ot, s0c : s0c + snc], ps[:, :snc]
                            )
                        for sti in range(snc // 128):
                            st = s0c // 128 + sti
                            ps = ps512.tile([128, 512], F32, tag="ps512")
                            for c in range(8):
                                nc.tensor.matmul(
                                    ps[:],
                                    xc_s[:, c, sti * 128 : (sti + 1) * 128],
                                    wv_sb[:, c, :],
                                    start=(c == 0),
                                    stop=(c == 7),
                                )
                            nc.scalar.copy(
                                v_aug[:, st, hf * 8 : (hf + 1) * 8, 0:64],
                                ps[:].rearrange("p (h e) -> p h e", e=64),
                            )

            # ---- phase A: attention + out-projection per qpair ----
            with (
                tc.tile_pool(name="expr", bufs=3) as exprpool,
                tc.tile_pool(name="expm", bufs=3) as expmpool,
                tc.tile_pool(name="recip", bufs=3) as recippool,
                tc.tile_pool(name="abl", bufs=3) as ablpool,
                tc.tile_pool(name="ctxs", bufs=2) as ctxpool,
                tc.tile_pool(name="ps_sc", bufs=2, space="PSUM") as ps_sc,
                tc.tile_pool(name="ps_ctx", bufs=2, space="PSUM") as ps_ctx,
            ):
                for qg in range(NQP // 2):
                  ctx_sb = ctxpool.tile(
                      [128, 8, 512], F16, name=f"ctx_{qg}", tag="ctx"
                  )
                  for qph in range(2):
                    qp = qg * 2 + qph
                    qsl = slice(qph * 256, qph * 256 + 256)
                    ms = 1 if qp == 0 else 0
                    for t in range(NH // 2):  # head pair
                        pss = [
                            ps_sc.tile(
                                [128, NKT, 256], F32,
                                name=f"sc_{qp}_{2 * t + par}", tag="sc",
                            )
                            for par in range(2)
                        ]
                        # raw scores (no mask inject: masking is a {0,1}
                        # multiply on the exp output instead)
                        for par in range(2):
                            hsl = slice(par * 64, par * 64 + 64)
                            for kt in range(NKT):
                                lj0 = qp * 256 + kt * 128
                                nc.tensor.matmul(
                                    pss[par][:, kt, :],
                                    kT_sb[hsl, t, lj0 : lj0 + 128],
                                    qT_sb[hsl, t, qp * 256 : qp * 256 + 256],
                                    start=True,
                                    stop=True,
                                    skip_group_check=True,
                                )
                        expms = []
                        for par in range(2):
                            h = 2 * t + par
                            expr_sb = exprpool.tile(
                                [128, NKT, 256], BF16,
                                name=f"er_{qp}_{h}", tag="expr",
                            )
                            nc.scalar.activation(
                                expr_sb[:], pss[par][:], EXP, bias=ebias[:]
                            )
                            expm_sb = expmpool.tile(
                                [128, NKT, 256], BF16,
                                name=f"em_{qp}_{h}", tag="expm",
                            )
                            nc.vector.tensor_mul(
                                expm_sb[:], expr_sb[:], mask_sb[:, ms, :, :]
                            )
                            expms.append(expm_sb)
                        psc = ps_ctx.tile(
                            [65, 2, 256], F32, name=f"ctxp_{qp}_{t}", tag="ctxp"
                        )
                        for par in range(2):
                            h = 2 * t + par
                            for kt in range(NKT):
                                nc.tensor.matmul(
                                    psc[:, par, :],
                                    v_sb[:, qp * 2 + kt, h * 65 : h * 65 + 65],
                                    expms[par][:, kt, :],
                                    start=(kt == 0),
                                    stop=(kt == NKT - 1),
                                )
                        # denominators (row 64) -> bf16 -> K=1 ones-MM
                        # broadcast to all 128 partitions -> wide reciprocal
                        rec = recippool.tile(
                            [1, 2, 256], BF16, name=f"rec_{qp}_{t}", tag="rec"
                        )
                        nc.vector.tensor_copy(rec[:], psc[64:65, :, :])
                        psb = ps512.tile(
                            [128, 256], F32, name=f"psb_{qp}_{t}", tag="ps512"
                        )
                        for par in range(2):
                            nc.tensor.matmul(
                                psb[:],
                                pmask[:, par, :],
                                rec[:, par, :],
                                start=(par == 0),
                                stop=(par == 1),
                            )
                        rb = recippool.tile(
                            [128, 256], F32, name=f"rb_{qp}_{t}", tag="rb"
                        )
                        nc.vector.reciprocal_approx_fast(rb[:], psb[:])
                        # drain pair to f32 scratch (raw ctx overflows fp16):
                        # even head -> parts 0:64, odd -> 64:128
                        cs32 = recippool.tile(
                            [128, 256], F32, name=f"cs_{qp}_{t}", tag="cs"
                        )
                        nc.vector.tensor_copy(cs32[0:64, :], psc[0:64, 0, :])
                        nc.vector.tensor_copy(cs32[64:128, :], psc[0:64, 1, :])
                        abl_sb = ablpool.tile(
                            [128, 256], F16, name=f"abl_{qp}_{t}", tag="abl"
                        )
                        nc.sync.dma_start(
                            abl_sb[:], ablT_d[:, t, qp * 256 : qp * 256 + 256]
                        )
                        # normalize on the write into fp16, then ablate
                        nc.vector.tensor_mul(ctx_sb[:, t, qsl], cs32[:], rb[:])
                        nc.vector.tensor_mul(
                            ctx_sb[:, t, qsl], ctx_sb[:, t, qsl], abl_sb[:]
                        )

                  # out projection for this pair of qpairs (N=512)
                  for ot in range(8):
                      wo_sb = wo_hs[ot // 4]
                      oi = ot % 4
                      ps = ps512.tile(
                          [128, 512], F32, name=f"op_{qg}_{ot}", tag="ps512"
                      )
                      for c in range(8):
                          nc.tensor.matmul(
                              ps[:],
                              wo_sb[:, c, oi * 128 : (oi + 1) * 128],
                              ctx_sb[:, c, :],
                              start=(c == 0),
                              stop=(c == 7),
                          )
                      o_sb = outpool.tile(
                          [128, 512], F32, name=f"out_{qg}_{ot}", tag="outp"
                      )
                      nc.scalar.activation(
                          o_sb[:], ps[:], IDENT, bias=bo_sb[:, ot : ot + 1]
                      )
                      nc.sync.dma_start(
                          outT_d[:, ot, qg * 512 : qg * 512 + 512], o_sb[:]
                      )
    nc.compile()
    return nc


def kernel(x, x_clean, ablation_mask, Wq, Wk, Wv, Wo, bo):
    global _compiled
    x = np.asarray(x, np.float16)
    x_clean = np.asarray(x_clean, np.float16)
    ablation_mask = np.asarray(ablation_mask, np.float16)
    WqT = np.ascontiguousarray(np.asarray(Wq, np.float16).T)
    WkT = np.ascontiguousarray(np.asarray(Wk, np.float16).T)
    WvT = np.ascontiguousarray(np.asarray(Wv, np.float16).T)
    WoT = np.ascontiguousarray(np.asarray(Wo, np.float16).T)
    bo = np.asarray(bo, np.float32)

    # pmask: routes even-head denominators to partitions 0:64, odd to 64:128
    pmask = np.zeros((1, 2, 128), np.float32)
    pmask[0, 0, 0:64] = 1.0
    pmask[0, 1, 64:128] = 1.0
    pmask = pmask.astype(ml_dtypes.bfloat16)

    # masks: include iff 1 <= kt*128 + r - a <= 256 ; set 1 adds qp==0 edge
    r = np.arange(128)[:, None]
    a = np.arange(256)[None, :]
    masks_by_half = []
    for half in range(2):
        m = np.empty((2, NKT, 128, 256), np.float32)
        for kt in range(NKT):
            d = kt * 128 + r - a
            inc = (d >= 1) & (d <= 256)
            m[0, kt] = np.where(inc, 1.0, 0.0)
            inc_edge = inc & ((kt * 128 + r) >= 256) if half == 0 else inc
            m[1, kt] = np.where(inc_edge, 1.0, 0.0)
        masks_by_half.append(m.astype(ml_dtypes.bfloat16))

    in_maps = []
    for c in range(NC):
        b, half = c // 2, c % 2
        s0 = half * SL
        xTc = np.ascontiguousarray(x[b, s0 : s0 + SL].T)
        xc = np.zeros((SKL, H), np.float16)
        lo = max(0, s0 - W)
        xc[W - (s0 - lo) :] = x_clean[b, lo : s0 + SL]
        xcTc = np.ascontiguousarray(xc.T)
        ablTc = np.ascontiguousarray(ablation_mask[b, s0 : s0 + SL].T)
        in_maps.append(
            {
                "xT": xTc,
                "xcT": xcTc,
                "ablT": ablTc,
                "WqT": WqT,
                "WkT": WkT,
                "WvT": WvT,
                "WoT": WoT,
                "bo": bo,
                "masks": masks_by_half[half],
                "pmask_in": pmask,
            }
        )

    if _compiled is None:
        _compiled = _build()
    res = run_bass_kernel_spmd(
        _compiled, in_maps, core_ids=list(range(NC)), trace=False
    )

    out = np.empty((B, S, H), np.float32)
    for c in range(NC):
        b, half = c // 2, c % 2
        out[b, half * SL : (half + 1) * SL] = res.results[c]["outT"].T
    return out


# revision 21
# speedup vs baseline: 1.0979x; 1.0979x over previous
"""AttentionWithSelfAblation TRN2 kernel.

Reference computation (B=4, S=2048, H=1024, nh=16, hd=64, window=256):
    q = x @ Wq.T ; k = x_clean @ Wk.T ; v = x_clean @ Wv.T   (per-head split)
    scores = q @ k.T  (NO 1/sqrt(hd) scaling)
    local causal mask: key j visible to query i iff i-255 <= j <= i
    attn = softmax(scores) ; ctx = attn @ v  (merge heads)
    out = (ctx * ablation_mask) @ Wo.T + bo
Sharding: pure data/sequence parallel over 8 cores: core c = (batch c//2,
sequence half c%2 of 1024 queries). Keys/values need a 256-halo to the left;
the first half uses zero-padding + masks instead. No collectives.

Dtypes: fp16 for x/xc/weights/qT/kT/ctx (scores accumulate fp32 in PSUM),
bf16 for v/exp (exp needs bf16 range: raw scores reach ~75; exp(s-20) bias
cancels in the softmax normalization). Measured end-to-end rel err ~3e-3.

Per-core device pipeline (all feature-major "T" layouts):
  phase Q : xT chunks streamed -> qT[o,s]
  phase KV: xcT chunks streamed -> kT[o,s] + v[s,o] (o augmented with a
            ones column per head: ctx matmul also yields the denominator)
  phase A : per (qpair of 256 queries, head pair): raw scoresT[sk,sq] by
            interleaved row-disjoint 64-row qk MMs (PE runs pairs
            concurrently); exp(s-20) on ACT -> bf16; {0,1} band mask
            multiply on DVE; ctx MMs split into 64-row halves into two
            PSUM banks (A/B) interleaved for PE concurrency; den =
            A[64]+B[64] -> ones-MM broadcast to 64 partitions -> DVE
            reciprocal; drain = (A+B) then *recip then *ablation -> fp16
            ctx; out-proj (PE, fp16) + bias (ACT).
Host does all layout transposes (free) and unshards by concatenation.
"""

import numpy as np
import ml_dtypes

from concourse import bacc
import concourse.tile as tile
import concourse.mybir as mybir
from concourse.bass_utils import run_bass_kernel_spmd

B, S, H = 4, 2048, 1024
NH, HD = 16, 64
W = 256  # window
SL = 1024  # per-core sequence chunk
SKL = SL + W  # keys incl halo
NQP = SL // 256  # qpairs of 256 queries
NKT = 4  # k-tiles of 128 per qpair
NC = 8  # cores

F32 = mybir.dt.float32
F32R = mybir.dt.float32r
F16 = mybir.dt.float16
BF16 = mybir.dt.bfloat16
EXP = mybir.ActivationFunctionType.Exp
IDENT = mybir.ActivationFunctionType.Identity
MULT = mybir.AluOpType.mult

EXP_BIAS = -20.0  # exp(s + EXP_BIAS): cancels in softmax, avoids overflow

_compiled = None


def _build():
    nc = bacc.Bacc("TRN2", target_bir_lowering=False, debug=False)

    xT = nc.dram_tensor("xT", [H, SL], F16, kind="ExternalInput")
    xcT = nc.dram_tensor("xcT", [H, SKL], F16, kind="ExternalInput")
    ablT = nc.dram_tensor("ablT", [H, SL], F16, kind="ExternalInput")
    WqT = nc.dram_tensor("WqT", [H, H], F16, kind="ExternalInput")
    WkT = nc.dram_tensor("WkT", [H, H], F16, kind="ExternalInput")
    WvT = nc.dram_tensor("WvT", [H, H], F16, kind="ExternalInput")
    WoT = nc.dram_tensor("WoT", [H, H], F16, kind="ExternalInput")
    bo = nc.dram_tensor("bo", [H], F32, kind="ExternalInput")
    # masks[set, kt, sk, sq] in {1,0}: set 0 = standard, set 1 = qp==0 variant
    masks = nc.dram_tensor("masks", [2, NKT, 128, 256], BF16, kind="ExternalInput")
    pmask_in = nc.dram_tensor("pmask_in", [1, 2, 128], BF16, kind="ExternalInput")
    outT = nc.dram_tensor("outT", [H, SL], F32, kind="ExternalOutput")

    xT_d = xT.rearrange("(c p) s -> p c s", p=128)
    xcT_d = xcT.rearrange("(c p) s -> p c s", p=128)
    ablT_d = ablT.rearrange("(t p) s -> p t s", p=128)
    outT_d = outT.rearrange("(t p) s -> p t s", p=128)

    with tile.TileContext(nc) as tc:
        with (
            tc.tile_pool(name="consts", bufs=1) as consts,
            tc.tile_pool(name="big", bufs=1) as big,
            tc.tile_pool(name="wpool", bufs=4) as wpool,
            tc.tile_pool(name="outp", bufs=3) as outpool,
            tc.tile_pool(name="ps512", bufs=2, space="PSUM") as ps512,
        ):

            qT_sb = big.tile([128, 8, SL], F16)
            kT_sb = big.tile([128, 8, SKL], F16)
            v_sb = big.tile([128, 10, 16 * 65], BF16)

            def load_weight_half(dram, hf):
                """o-columns [hf*512, (hf+1)*512) of a transposed weight.
                Issued on the scalar (qActDynamicHW) DMA ring so weight
                prefetch never head-of-line-blocks the x/xc chunk stream
                on the sync ring."""
                w_sb = wpool.tile(
                    [128, 8, 512], F16, name=f"w_{dram.name}_{hf}", tag="w"
                )
                for c in range(8):
                    nc.scalar.dma_start(
                        w_sb[:, c, :],
                        dram.rearrange("(c p) o -> p c o", p=128)[
                            :, c, hf * 512 : (hf + 1) * 512
                        ],
                    )
                return w_sb

            # ones columns of the augmented v (slot 64 of each head's 65):
            # memset a contiguous scratch then strided-copy into place
            v_aug = v_sb[:].rearrange("p t (h e) -> p t h e", e=65)
            ones_scratch = consts.tile([128, 160], BF16)
            nc.vector.memset(ones_scratch[:], 1.0)
            nc.vector.tensor_copy(
                v_aug[:, :, :, 64],
                ones_scratch[:].rearrange("p (t h) -> p t h", t=10),
            )
            # per-partition bias column for exp(s + EXP_BIAS)
            ebias = consts.tile([128, 1], F32)
            nc.vector.memset(ebias[:], EXP_BIAS)

            # ---- phase Q with scoped projection PSUM pool ----
            # All weight halves fit in SBUF at fp16 (bufs=8) and ride the
            # scalar DMA ring, so prefetch everything upfront.
            wq_hs = [load_weight_half(WqT, hf) for hf in range(2)]
            # attention constants: also on the scalar ring, early
            bo_sb = consts.tile([128, 8], F32)
            nc.scalar.dma_start(bo_sb[:], bo.rearrange("(t p) -> p t", p=128))
            pmask = consts.tile([1, 2, 128], BF16)
            nc.scalar.dma_start(pmask[:], pmask_in[:])
            mask_sb = consts.tile([128, 2, NKT, 256], BF16)
            for ms in range(2):
                nc.scalar.dma_start(
                    mask_sb[:, ms, :, :],
                    masks.rearrange("s t k q -> k s t q")[:, ms, :, :],
                )
            wk_hs = [load_weight_half(WkT, hf) for hf in range(2)]
            wv_hs = [load_weight_half(WvT, hf) for hf in range(2)]
            wo_hs = [load_weight_half(WoT, hf) for hf in range(2)]
            with tc.tile_pool(name="xs", bufs=2) as xspool:
                for hf in range(2):
                    wq_sb = wq_hs[hf]
                    for ci in range(SL // 512):
                        x_s = xspool.tile(
                            [128, 8, 512], F16, name=f"x_{hf}_{ci}", tag="xs"
                        )
                        for c in range(8):
                            nc.sync.dma_start(
                                x_s[:, c, :], xT_d[:, c, ci * 512 : (ci + 1) * 512]
                            )
                        for oi in range(4):
                            ot = hf * 4 + oi
                            ps = ps512.tile([128, 512], F32, tag="ps512")
                            for c in range(8):
                                nc.tensor.matmul(
                                    ps[:],
                                    wq_sb[:, c, oi * 128 : (oi + 1) * 128],
                                    x_s[:, c, :],
                                    start=(c == 0),
                                    stop=(c == 7),
                                )
                            nc.vector.tensor_copy(
                                qT_sb[:, ot, ci * 512 : (ci + 1) * 512], ps[:]
                            )

            # ---- phase KV: kT[o, s] + v[s, o] (o augmented per head) ----
            kv_chunks = [(0, 512), (512, 512), (1024, 256)]
            with tc.tile_pool(name="xcs", bufs=2) as xcspool:
                for hf in range(2):
                    wk_sb = wk_hs[hf]
                    wv_sb = wv_hs[hf]
                    for ci, (s0c, snc) in enumerate(kv_chunks):
                        xc_s = xcspool.tile(
                            [128, 8, 512], F16, name=f"xc_{hf}_{ci}", tag="xcs"
                        )
                        for c in range(8):
                            nc.sync.dma_start(
                                xc_s[:, c, :snc], xcT_d[:, c, s0c : s0c + snc]
                            )
                        for oi in range(4):
                            ot = hf * 4 + oi
                            ps = ps512.tile([128, 512], F32, tag="ps512")
                            for c in range(8):
                                nc.tensor.matmul(
                                    ps[:, :snc],
                                    wk_sb[:, c, oi * 128 : (oi + 1) * 128],
                                    xc_s[:, c, :snc],
                                    start=(c == 0),
                                    stop=(c == 7),
                                )
                            nc.vector.tensor_copy(
                                kT_sb[:, ot, s0c : s0c + snc], ps[:, :snc]
                            )
                        for sti in range(snc // 128):
                            st = s0c // 128 + sti
                            ps = ps512.tile([128, 512], F32, tag="ps512")
                            for c in range(8):
                                nc.tensor.matmul(
                                    ps[:],
                                    xc_s[:, c, sti * 128 : (sti + 1) * 128],
                                    wv_sb[:, c, :],
                                    start=(c == 0),
                                    stop=(c == 7),
                                )
                            nc.scalar.copy(
                                v_aug[:, st, hf * 8 : (hf + 1) * 8, 0:64],
                                ps[:].rearrange("p (h e) -> p h e", e=64),
                            )

            # ---- phase A: attention + out-projection per qpair ----
            with (
                tc.tile_pool(name="expr", bufs=3) as exprpool,
                tc.tile_pool(name="expm", bufs=3) as expmpool,
                tc.tile_pool(name="recip", bufs=3) as recippool,
                tc.tile_pool(name="abl", bufs=3) as ablpool,
                tc.tile_pool(name="ctxs", bufs=2) as ctxpool,
                tc.tile_pool(name="ps_sc", bufs=2, space="PSUM") as ps_sc,
                tc.tile_pool(name="ps_ctx", bufs=2, space="PSUM") as ps_ctx,
            ):
                def emit_outproj(ctx_tile, qg, ot):
                    """One out-projection group (N=512). Interleaved between
                    head iterations of the NEXT query group as dependency-free
                    PE filler: absorbs exp/mask-latency stalls and keeps the
                    PE clock (HAM) warm through phase A."""
                    wo_sb = wo_hs[ot // 4]
                    oi = ot % 4
                    ps = ps512.tile(
                        [128, 512], F32, name=f"op_{qg}_{ot}", tag="ps512"
                    )
                    for c in range(8):
                        nc.tensor.matmul(
                            ps[:],
                            wo_sb[:, c, oi * 128 : (oi + 1) * 128],
                            ctx_tile[:, c, :],
                            start=(c == 0),
                            stop=(c == 7),
                        )
                    o_sb = outpool.tile(
                        [128, 512], F32, name=f"out_{qg}_{ot}", tag="outp"
                    )
                    nc.scalar.activation(
                        o_sb[:], ps[:], IDENT, bias=bo_sb[:, ot : ot + 1]
                    )
                    nc.sync.dma_start(
                        outT_d[:, ot, qg * 512 : qg * 512 + 512], o_sb[:]
                    )

                fillers = []
                for qg in range(NQP // 2):
                  ctx_sb = ctxpool.tile(
                      [128, 8, 512], F16, name=f"ctx_{qg}", tag="ctx"
                  )
                  for qph in range(2):
                    qp = qg * 2 + qph
                    qsl = slice(qph * 256, qph * 256 + 256)
                    ms = 1 if qp == 0 else 0
                    for t in range(NH // 2):  # head pair
                        pss = [
                            ps_sc.tile(
                                [128, NKT, 256], F32,
                                name=f"sc_{qp}_{2 * t + par}", tag="sc",
                            )
                            for par in range(2)
                        ]
                        # raw scores (no mask inject: masking is a {0,1}
                        # multiply on the exp output instead)
                        for par in range(2):
                            hsl = slice(par * 64, par * 64 + 64)
                            for kt in range(NKT):
                                lj0 = qp * 256 + kt * 128
                                nc.tensor.matmul(
                                    pss[par][:, kt, :],
                                    kT_sb[hsl, t, lj0 : lj0 + 128],
                                    qT_sb[hsl, t, qp * 256 : qp * 256 + 256],
                                    start=True,
                                    stop=True,
                                    skip_group_check=True,
                                )
                        # dependency-free PE filler while ACT/DVE produce expm
                        if fillers:
                            emit_outproj(*fillers.pop(0))
                        expms = []
                        for par in range(2):
                            h = 2 * t + par
                            expr_sb = exprpool.tile(
                                [128, NKT, 256], BF16,
                                name=f"er_{qp}_{h}", tag="expr",
                            )
                            nc.scalar.activation(
                                expr_sb[:], pss[par][:], EXP, bias=ebias[:]
                            )
                            expm_sb = expmpool.tile(
                                [128, NKT, 256], BF16,
                                name=f"em_{qp}_{h}", tag="expm",
                            )
                            nc.vector.tensor_mul(
                                expm_sb[:], expr_sb[:], mask_sb[:, ms, :, :]
                            )
                            expms.append(expm_sb)
                        psc = ps_ctx.tile(
                            [65, 2, 256], F32, name=f"ctxp_{qp}_{t}", tag="ctxp"
                        )
                        for par in range(2):
                            h = 2 * t + par
                            for kt in range(NKT):
                                nc.tensor.matmul(
                                    psc[:, par, :],
                                    v_sb[:, qp * 2 + kt, h * 65 : h * 65 + 65],
                                    expms[par][:, kt, :],
                                    start=(kt == 0),
                                    stop=(kt == NKT - 1),
                                )
                        # denominators (row 64) -> bf16 -> K=1 ones-MM
                        # broadcast to all 128 partitions -> wide reciprocal
                        rec = recippool.tile(
                            [1, 2, 256], BF16, name=f"rec_{qp}_{t}", tag="rec"
                        )
                        nc.vector.tensor_copy(rec[:], psc[64:65, :, :])
                        psb = ps512.tile(
                            [128, 256], F32, name=f"psb_{qp}_{t}", tag="ps512"
                        )
                        for par in range(2):
                            nc.tensor.matmul(
                                psb[:],
                                pmask[:, par, :],
                                rec[:, par, :],
                                start=(par == 0),
                                stop=(par == 1),
                            )
                        rb = recippool.tile(
                            [128, 256], F32, name=f"rb_{qp}_{t}", tag="rb"
                        )
                        nc.vector.reciprocal_approx_fast(rb[:], psb[:])
                        # drain pair to f32 scratch (raw ctx overflows fp16):
                        # even head -> parts 0:64, odd -> 64:128
                        cs32 = recippool.tile(
                            [128, 256], F32, name=f"cs_{qp}_{t}", tag="cs"
                        )
                        nc.vector.tensor_copy(cs32[0:64, :], psc[0:64, 0, :])
                        nc.vector.tensor_copy(cs32[64:128, :], psc[0:64, 1, :])
                        abl_sb = ablpool.tile(
                            [128, 256], F16, name=f"abl_{qp}_{t}", tag="abl"
                        )
                        nc.sync.dma_start(
                            abl_sb[:], ablT_d[:, t, qp * 256 : qp * 256 + 256]
                        )
                        # normalize on the write into fp16, then ablate —
                        # on GpSimd (all-SBUF operands, engine otherwise idle)
                        nc.gpsimd.tensor_mul(ctx_sb[:, t, qsl], cs32[:], rb[:])
                        nc.gpsimd.tensor_mul(
                            ctx_sb[:, t, qsl], ctx_sb[:, t, qsl], abl_sb[:]
                        )

                  # out projection groups become filler in the next qg
                  fillers += [(ctx_sb, qg, ot) for ot in range(8)]
                for f in fillers:
                    emit_outproj(*f)
    nc.compile()
    return nc


def kernel(x, x_clean, ablation_mask, Wq, Wk, Wv, Wo, bo):
    global _compiled
    x = np.asarray(x, np.float16)
    x_clean = np.asarray(x_clean, np.float16)
    ablation_mask = np.asarray(ablation_mask, np.float16)
    WqT = np.ascontiguousarray(np.asarray(Wq, np.float16).T)
    WkT = np.ascontiguousarray(np.asarray(Wk, np.float16).T)
    WvT = np.ascontiguousarray(np.asarray(Wv, np.float16).T)
    WoT = np.ascontiguousarray(np.asarray(Wo, np.float16).T)
    bo = np.asarray(bo, np.float32)

    # pmask: routes even-head denominators to partitions 0:64, odd to 64:128
    pmask = np.zeros((1, 2, 128), np.float32)
    pmask[0, 0, 0:64] = 1.0
    pmask[0, 1, 64:128] = 1.0
    pmask = pmask.astype(ml_dtypes.bfloat16)

    # masks: include iff 1 <= kt*128 + r - a <= 256 ; set 1 adds qp==0 edge
    r = np.arange(128)[:, None]
    a = np.arange(256)[None, :]
    masks_by_half = []
    for half in range(2):
        m = np.empty((2, NKT, 128, 256), np.float32)
        for kt in range(NKT):
            d = kt * 128 + r - a
            inc = (d >= 1) & (d <= 256)
            m[0, kt] = np.where(inc, 1.0, 0.0)
            inc_edge = inc & ((kt * 128 + r) >= 256) if half == 0 else inc
            m[1, kt] = np.where(inc_edge, 1.0, 0.0)
        masks_by_half.append(m.astype(ml_dtypes.bfloat16))

    in_maps = []
    for c in range(NC):
        b, half = c // 2, c % 2
        s0 = half * SL
        xTc = np.ascontiguousarray(x[b, s0 : s0 + SL].T)
        xc = np.zeros((SKL, H), np.float16)
        lo = max(0, s0 - W)
        xc[W - (s0 - lo) :] = x_clean[b, lo : s0 + SL]
        xcTc = np.ascontiguousarray(xc.T)
        ablTc = np.ascontiguousarray(ablation_mask[b, s0 : s0 + SL].T)
        in_maps.append(
            {
                "xT": xTc,
                "xcT": xcTc,
                "ablT": ablTc,
                "WqT": WqT,
                "WkT": WkT,
                "WvT": WvT,
                "WoT": WoT,
                "bo": bo,
                "masks": masks_by_half[half],
                "pmask_in": pmask,
            }
        )

    if _compiled is None:
        _compiled = _build()
    res = run_bass_kernel_spmd(
        _compiled, in_maps, core_ids=list(range(NC)), trace=False
    )

    out = np.empty((B, S, H), np.float32)
    for c in range(NC):
        b, half = c // 2, c % 2
        out[b, half * SL : (half + 1) * SL] = res.results[c]["outT"].T
    return out


# revision 31
# speedup vs baseline: 1.2422x; 1.1315x over previous
"""AttentionWithSelfAblation TRN2 kernel.

Reference computation (B=4, S=2048, H=1024, nh=16, hd=64, window=256):
    q = x @ Wq.T ; k = x_clean @ Wk.T ; v = x_clean @ Wv.T   (per-head split)
    scores = q @ k.T  (NO 1/sqrt(hd) scaling)
    local causal mask: key j visible to query i iff i-255 <= j <= i
    attn = softmax(scores) ; ctx = attn @ v  (merge heads)
    out = (ctx * ablation_mask) @ Wo.T + bo
Sharding: pure data/sequence parallel over 8 cores: core c = (batch c//2,
sequence half c%2 of 1024 queries). Keys/values need a 256-halo to the left;
the first half uses zero-padding + masks instead. No collectives.

Dtypes: fp16 for x/xc/weights/qT/kT/ctx (scores accumulate fp32 in PSUM),
bf16 for v/exp (exp needs bf16 range: raw scores reach ~75; exp(s-20) bias
cancels in the softmax normalization). Measured end-to-end rel err ~3e-3.

Per-core device pipeline (all feature-major "T" layouts):
  phase Q : xT chunks streamed -> qT[o,s]
  phase KV: xcT chunks streamed -> kT[o,s] + v[s,o] (o augmented with a
            ones column per head: ctx matmul also yields the denominator)
  phase A : per (qpair of 256 queries, head pair): raw scoresT[sk,sq] by
            interleaved row-disjoint 64-row qk MMs (PE runs pairs
            concurrently); exp(s-20) on ACT -> bf16; {0,1} band mask
            multiply on DVE; ctx MMs split into 64-row halves into two
            PSUM banks (A/B) interleaved for PE concurrency; den =
            A[64]+B[64] -> ones-MM broadcast to 64 partitions -> DVE
            reciprocal; drain = (A+B) then *recip then *ablation -> fp16
            ctx; out-proj (PE, fp16) + bias (ACT).
Host does all layout transposes (free) and unshards by concatenation.
"""

import numpy as np
import ml_dtypes

from concourse import bacc
import concourse.tile as tile
import concourse.mybir as mybir
from concourse.bass_utils import run_bass_kernel_spmd

B, S, H = 4, 2048, 1024
NH, HD = 16, 64
W = 256  # window
SL = 1024  # per-core sequence chunk
SKL = SL + W  # keys incl halo
NQP = SL // 256  # qpairs of 256 queries
NKT = 4  # k-tiles of 128 per qpair
NC = 8  # cores

F32 = mybir.dt.float32
F32R = mybir.dt.float32r
F16 = mybir.dt.float16
BF16 = mybir.dt.bfloat16
EXP = mybir.ActivationFunctionType.Exp
IDENT = mybir.ActivationFunctionType.Identity
MULT = mybir.AluOpType.mult

EXP_BIAS = -20.0  # exp(s + EXP_BIAS): cancels in softmax, avoids overflow

_compiled = None


def _build():
    nc = bacc.Bacc("TRN2", target_bir_lowering=False, debug=False)

    xT = nc.dram_tensor("xT", [H, SL], F16, kind="ExternalInput")
    xcT = nc.dram_tensor("xcT", [H, SKL], F16, kind="ExternalInput")
    ablT = nc.dram_tensor("ablT", [H, SL], F16, kind="ExternalInput")
    WqT = nc.dram_tensor("WqT", [H, H], F16, kind="ExternalInput")
    WkT = nc.dram_tensor("WkT", [H, H], F16, kind="ExternalInput")
    WvT = nc.dram_tensor("WvT", [H, H], F16, kind="ExternalInput")
    WoT = nc.dram_tensor("WoT", [H, H], F16, kind="ExternalInput")
    bo = nc.dram_tensor("bo", [H], F32, kind="ExternalInput")
    # masks[set, kt, sk, sq] additive {0,-1e30}: set 1 = qp==0 variant
    masks = nc.dram_tensor("masks", [2, NKT, 128, 256], BF16, kind="ExternalInput")
    ident_in = nc.dram_tensor("ident_in", [128, 128], BF16, kind="ExternalInput")
    pmask_in = nc.dram_tensor("pmask_in", [1, 2, 128], BF16, kind="ExternalInput")
    outT = nc.dram_tensor("outT", [H, SL], F32, kind="ExternalOutput")

    xT_d = xT.rearrange("(c p) s -> p c s", p=128)
    xcT_d = xcT.rearrange("(c p) s -> p c s", p=128)
    ablT_d = ablT.rearrange("(t p) s -> p t s", p=128)
    outT_d = outT.rearrange("(t p) s -> p t s", p=128)

    with tile.TileContext(nc) as tc:
        with (
            tc.tile_pool(name="consts", bufs=1) as consts,
            tc.tile_pool(name="big", bufs=1) as big,
            tc.tile_pool(name="wpool", bufs=4) as wpool,
            tc.tile_pool(name="outp", bufs=3) as outpool,
            tc.tile_pool(name="ps512", bufs=2, space="PSUM") as ps512,
        ):

            qT_sb = big.tile([128, 8, SL], F16)
            kT_sb = big.tile([128, 8, SKL], F16)
            v_sb = big.tile([128, 10, 16 * 65], BF16)

            def load_weight_half(dram, hf):
                """o-columns [hf*512, (hf+1)*512) of a transposed weight.
                One batched dma_start (issue costs ~0.6us of serial
                sequencer time each) on the scalar (qActDynamicHW) ring so
                weight prefetch never blocks the x/xc stream on sync."""
                w_sb = wpool.tile(
                    [128, 8, 512], F16, name=f"w_{dram.name}_{hf}", tag="w"
                )
                nc.scalar.dma_start(
                    w_sb[:],
                    dram.rearrange("(c p) o -> p c o", p=128)[
                        :, :, hf * 512 : (hf + 1) * 512
                    ],
                )
                return w_sb

            # ones columns of the augmented v (slot 64 of each head's 65):
            # memset a contiguous scratch then strided-copy into place
            v_aug = v_sb[:].rearrange("p t (h e) -> p t h e", e=65)
            ones_scratch = consts.tile([128, 160], BF16)
            nc.vector.memset(ones_scratch[:], 1.0)
            nc.vector.tensor_copy(
                v_aug[:, :, :, 64],
                ones_scratch[:].rearrange("p (t h) -> p t h", t=10),
            )
            # per-partition bias column for exp(s + EXP_BIAS)
            ebias = consts.tile([128, 1], F32)
            nc.vector.memset(ebias[:], EXP_BIAS)

            # ---- phase Q with scoped projection PSUM pool ----
            # All weight halves fit in SBUF at fp16 (bufs=8) and ride the
            # scalar DMA ring, so prefetch everything upfront.
            wq_hs = [load_weight_half(WqT, hf) for hf in range(2)]
            # attention constants: also on the scalar ring, early
            bo_sb = consts.tile([128, 8], F32)
            nc.scalar.dma_start(bo_sb[:], bo.rearrange("(t p) -> p t", p=128))
            pmask = consts.tile([1, 2, 128], BF16)
            nc.scalar.dma_start(pmask[:], pmask_in[:])
            ident = consts.tile([128, 128], BF16)
            nc.scalar.dma_start(ident[:], ident_in[:])
            mask_sb = consts.tile([128, 2, NKT, 256], BF16)
            nc.scalar.dma_start(
                mask_sb[:], masks.rearrange("s t k q -> k s t q")
            )
            wk_hs = [load_weight_half(WkT, hf) for hf in range(2)]
            wv_hs = [load_weight_half(WvT, hf) for hf in range(2)]
            wo_hs = [load_weight_half(WoT, hf) for hf in range(2)]
            with tc.tile_pool(name="xs", bufs=2) as xspool:
                for hf in range(2):
                    wq_sb = wq_hs[hf]
                    for ci in range(SL // 512):
                        x_s = xspool.tile(
                            [128, 8, 512], F16, name=f"x_{hf}_{ci}", tag="xs"
                        )
                        nc.sync.dma_start(
                            x_s[:], xT_d[:, :, ci * 512 : (ci + 1) * 512]
                        )
                        for oi in range(4):
                            ot = hf * 4 + oi
                            ps = ps512.tile([128, 512], F32, tag="ps512")
                            for c in range(8):
                                nc.tensor.matmul(
                                    ps[:],
                                    wq_sb[:, c, oi * 128 : (oi + 1) * 128],
                                    x_s[:, c, :],
                                    start=(c == 0),
                                    stop=(c == 7),
                                )
                            nc.vector.tensor_copy(
                                qT_sb[:, ot, ci * 512 : (ci + 1) * 512], ps[:]
                            )

            # ---- phase KV: kT[o, s] + v[s, o] (o augmented per head) ----
            kv_chunks = [(0, 512), (512, 512), (1024, 256)]
            with tc.tile_pool(name="xcs", bufs=2) as xcspool:
                for hf in range(2):
                    wk_sb = wk_hs[hf]
                    wv_sb = wv_hs[hf]
                    for ci, (s0c, snc) in enumerate(kv_chunks):
                        xc_s = xcspool.tile(
                            [128, 8, 512], F16, name=f"xc_{hf}_{ci}", tag="xcs"
                        )
                        nc.sync.dma_start(
                            xc_s[:, :, :snc], xcT_d[:, :, s0c : s0c + snc]
                        )
                        for oi in range(4):
                            ot = hf * 4 + oi
                            ps = ps512.tile([128, 512], F32, tag="ps512")
                            for c in range(8):
                                nc.tensor.matmul(
                                    ps[:, :snc],
                                    wk_sb[:, c, oi * 128 : (oi + 1) * 128],
                                    xc_s[:, c, :snc],
                                    start=(c == 0),
                                    stop=(c == 7),
                                )
                            nc.vector.tensor_copy(
                                kT_sb[:, ot, s0c : s0c + snc], ps[:, :snc]
                            )
                        for sti in range(snc // 128):
                            st = s0c // 128 + sti
                            ps = ps512.tile([128, 512], F32, tag="ps512")
                            for c in range(8):
                                nc.tensor.matmul(
                                    ps[:],
                                    xc_s[:, c, sti * 128 : (sti + 1) * 128],
                                    wv_sb[:, c, :],
                                    start=(c == 0),
                                    stop=(c == 7),
                                )
                            nc.scalar.copy(
                                v_aug[:, st, hf * 8 : (hf + 1) * 8, 0:64],
                                ps[:].rearrange("p (h e) -> p h e", e=64),
                            )

            # ---- phase A: attention + out-projection per qpair ----
            with (
                tc.tile_pool(name="expr", bufs=3) as exprpool,
                tc.tile_pool(name="recip", bufs=3) as recippool,
                tc.tile_pool(name="abl", bufs=3) as ablpool,
                tc.tile_pool(name="ctxs", bufs=2) as ctxpool,
                tc.tile_pool(name="ps_sc", bufs=2, space="PSUM") as ps_sc,
                tc.tile_pool(name="ps_ctx", bufs=2, space="PSUM") as ps_ctx,
            ):
                def emit_outproj(ctx_tile, qg, ot):
                    """One out-projection group (N=512). Interleaved between
                    head iterations of the NEXT query group as dependency-free
                    PE filler: absorbs exp/mask-latency stalls and keeps the
                    PE clock (HAM) warm through phase A."""
                    wo_sb = wo_hs[ot // 4]
                    oi = ot % 4
                    ps = ps512.tile(
                        [128, 512], F32, name=f"op_{qg}_{ot}", tag="ps512"
                    )
                    for c in range(8):
                        nc.tensor.matmul(
                            ps[:],
                            wo_sb[:, c, oi * 128 : (oi + 1) * 128],
                            ctx_tile[:, c, :],
                            start=(c == 0),
                            stop=(c == 7),
                        )
                    o_sb = outpool.tile(
                        [128, 512], F32, name=f"out_{qg}_{ot}", tag="outp"
                    )
                    nc.scalar.activation(
                        o_sb[:], ps[:], IDENT, bias=bo_sb[:, ot : ot + 1]
                    )
                    nc.sync.dma_start(
                        outT_d[:, ot, qg * 512 : qg * 512 + 512], o_sb[:]
                    )

                fillers = []
                for qg in range(NQP // 2):
                  ctx_sb = ctxpool.tile(
                      [128, 8, 512], F16, name=f"ctx_{qg}", tag="ctx"
                  )
                  for qph in range(2):
                    qp = qg * 2 + qph
                    qsl = slice(qph * 256, qph * 256 + 256)
                    ms = 1 if qp == 0 else 0
                    # ablation tiles for all 8 h-tiles of this qpair, batched
                    abl_q = ablpool.tile(
                        [128, 8, 256], F16, name=f"abl_{qp}", tag="abl"
                    )
                    nc.sync.dma_start(
                        abl_q[:], ablT_d[:, :, qp * 256 : qp * 256 + 256]
                    )
                    for t in range(NH // 2):  # head pair
                        pss = [
                            ps_sc.tile(
                                [128, NKT, 256], F32,
                                name=f"sc_{qp}_{2 * t + par}", tag="sc",
                            )
                            for par in range(2)
                        ]
                        # PE-injected additive mask, then scores accumulate
                        for par in range(2):
                            for kg in range(2):  # one inject per 2 k-tiles
                                nc.tensor.matmul(
                                    pss[par][:, kg * 2 : kg * 2 + 2, :],
                                    ident[:],
                                    mask_sb[:, ms, kg * 2 : kg * 2 + 2, :],
                                    start=True,
                                    stop=False,
                                    skip_group_check=True,
                                )
                        for par in range(2):
                            hsl = slice(par * 64, par * 64 + 64)
                            for kt in range(NKT):
                                lj0 = qp * 256 + kt * 128
                                nc.tensor.matmul(
                                    pss[par][:, kt, :],
                                    kT_sb[hsl, t, lj0 : lj0 + 128],
                                    qT_sb[hsl, t, qp * 256 : qp * 256 + 256],
                                    start=False,
                                    stop=True,
                                    skip_group_check=True,
                                )
                        # dependency-free PE filler while ACT computes exp
                        if fillers:
                            emit_outproj(*fillers.pop(0))
                        exprs = []
                        for par in range(2):
                            h = 2 * t + par
                            expr_sb = exprpool.tile(
                                [128, NKT, 256], BF16,
                                name=f"er_{qp}_{h}", tag="expr",
                            )
                            nc.scalar.activation(
                                expr_sb[:], pss[par][:], EXP, bias=ebias[:]
                            )
                            exprs.append(expr_sb)
                        psc = ps_ctx.tile(
                            [65, 2, 256], F32, name=f"ctxp_{qp}_{t}", tag="ctxp"
                        )
                        for par in range(2):
                            h = 2 * t + par
                            for kt in range(NKT):
                                nc.tensor.matmul(
                                    psc[:, par, :],
                                    v_sb[:, qp * 2 + kt, h * 65 : h * 65 + 65],
                                    exprs[par][:, kt, :],
                                    start=(kt == 0),
                                    stop=(kt == NKT - 1),
                                )
                        # denominators (row 64) -> bf16 -> K=1 ones-MM
                        # broadcast to all 128 partitions -> wide reciprocal
                        rec = recippool.tile(
                            [1, 2, 256], BF16, name=f"rec_{qp}_{t}", tag="rec"
                        )
                        nc.vector.tensor_copy(rec[:], psc[64:65, :, :])
                        psb = ps512.tile(
                            [128, 256], F32, name=f"psb_{qp}_{t}", tag="ps512"
                        )
                        for par in range(2):
                            nc.tensor.matmul(
                                psb[:],
                                pmask[:, par, :],
                                rec[:, par, :],
                                start=(par == 0),
                                stop=(par == 1),
                            )
                        rb = recippool.tile(
                            [128, 256], F32, name=f"rb_{qp}_{t}", tag="rb"
                        )
                        nc.vector.reciprocal_approx_fast(rb[:], psb[:])
                        # drain pair to f32 scratch (raw ctx overflows fp16):
                        # even head -> parts 0:64, odd -> 64:128
                        cs32 = recippool.tile(
                            [128, 256], F32, name=f"cs_{qp}_{t}", tag="cs"
                        )
                        nc.vector.tensor_copy(cs32[0:64, :], psc[0:64, 0, :])
                        nc.vector.tensor_copy(cs32[64:128, :], psc[0:64, 1, :])
                        # normalize on the write into fp16, then ablate —
                        # on GpSimd (all-SBUF operands, engine otherwise idle)
                        nc.gpsimd.tensor_mul(ctx_sb[:, t, qsl], cs32[:], rb[:])
                        nc.gpsimd.tensor_mul(
                            ctx_sb[:, t, qsl], ctx_sb[:, t, qsl], abl_q[:, t, :]
                        )

                  # out projection groups become filler in the next qg
                  fillers += [(ctx_sb, qg, ot) for ot in range(8)]
                for f in fillers:
                    emit_outproj(*f)
    nc.compile()
    return nc


def kernel(x, x_clean, ablation_mask, Wq, Wk, Wv, Wo, bo):
    global _compiled
    x = np.asarray(x, np.float16)
    x_clean = np.asarray(x_clean, np.float16)
    ablation_mask = np.asarray(ablation_mask, np.float16)
    WqT = np.ascontiguousarray(np.asarray(Wq, np.float16).T)
    WkT = np.ascontiguousarray(np.asarray(Wk, np.float16).T)
    WvT = np.ascontiguousarray(np.asarray(Wv, np.float16).T)
    WoT = np.ascontiguousarray(np.asarray(Wo, np.float16).T)
    bo = np.asarray(bo, np.float32)

    # pmask: routes even-head denominators to partitions 0:64, odd to 64:128
    pmask = np.zeros((1, 2, 128), np.float32)
    pmask[0, 0, 0:64] = 1.0
    pmask[0, 1, 64:128] = 1.0
    pmask = pmask.astype(ml_dtypes.bfloat16)
    ident = np.eye(128, dtype=ml_dtypes.bfloat16)

    # masks: include iff 1 <= kt*128 + r - a <= 256 ; set 1 adds qp==0 edge
    r = np.arange(128)[:, None]
    a = np.arange(256)[None, :]
    masks_by_half = []
    for half in range(2):
        m = np.empty((2, NKT, 128, 256), np.float32)
        for kt in range(NKT):
            d = kt * 128 + r - a
            inc = (d >= 1) & (d <= 256)
            m[0, kt] = np.where(inc, 0.0, -1e30)
            inc_edge = inc & ((kt * 128 + r) >= 256) if half == 0 else inc
            m[1, kt] = np.where(inc_edge, 0.0, -1e30)
        masks_by_half.append(m.astype(ml_dtypes.bfloat16))

    in_maps = []
    for c in range(NC):
        b, half = c // 2, c % 2
        s0 = half * SL
        xTc = np.ascontiguousarray(x[b, s0 : s0 + SL].T)
        xc = np.zeros((SKL, H), np.float16)
        lo = max(0, s0 - W)
        xc[W - (s0 - lo) :] = x_clean[b, lo : s0 + SL]
        xcTc = np.ascontiguousarray(xc.T)
        ablTc = np.ascontiguousarray(ablation_mask[b, s0 : s0 + SL].T)
        in_maps.append(
            {
                "xT": xTc,
                "xcT": xcTc,
                "ablT": ablTc,
                "WqT": WqT,
                "WkT": WkT,
                "WvT": WvT,
                "WoT": WoT,
                "bo": bo,
                "masks": masks_by_half[half],
                "pmask_in": pmask,
                "ident_in": ident,
            }
        )

    if _compiled is None:
        _compiled = _build()
    res = run_bass_kernel_spmd(
        _compiled, in_maps, core_ids=list(range(NC)), trace=False
    )

    out = np.empty((B, S, H), np.float32)
    for c in range(NC):
        b, half = c // 2, c % 2
        out[b, half * SL : (half + 1) * SL] = res.results[c]["outT"].T
    return out
